# revision 7
# baseline (speedup 1.0000x reference)
"""Linear attention kernel for 8 Trainium2 NeuronCores.

Sharding: core = 2*b + hg  (b in 0..3 batches, hg in 0..1 head-groups of 8 heads).
Fully data-parallel — no collectives; host sums the two head-group partials per
batch. Each core adds bias/2 so the pair-sum carries the full bias.

Per-core math (T=4096 tokens, CH=512 = 8 heads x 64, DIM=1024):
  qT   = (x @ Wq)^T          c-major [CH, T], elu+1
  k,v  = x @ Wk, x @ Wv      token-major [T, CH], elu+1 on k
  kvT  = v^T k  (per head-pair, diagonal 64-blocks valid), accumulated in PSUM
  z    = ones^T k            [1, CH], accumulated in PSUM
  den  = Zblock^T qT         [8, T]   (Zblock = block-diag expansion of z)
  r    = 1/(den + 1e-6); rbc = E^T r  (broadcast r over each head's 64 rows)
  qsc  = qT * rbc
  M    = kvT^T @ W2  (per 128-row ch-tile; off-diag blocks of kvT zeroed)
  y    = qsc^T @ M + bias/2  token-major [T, DIM]
"""

import sys

sys.path.insert(0, "/opt/trn_rl_repo")

import numpy as np

import concourse.bass as bass
import concourse.mybir as mybir
import concourse.tile as tile
from concourse import bacc

F32 = mybir.dt.float32
AF = mybir.ActivationFunctionType

DIM = 1024      # model dim (contraction for projections)
CH = 512        # per-core channels (8 heads x 64)
P = 128

N_CORES = 8
B, T_FULL = 4, 4096


def build_nc(T=T_FULL):
    NTB = T // 512          # 512-token blocks
    nc = bacc.Bacc(None, target_bir_lowering=False, debug=False)

    xT = nc.declare_dram_parameter("xT", [DIM, T], F32, isOutput=False)
    w1 = nc.declare_dram_parameter("w1", [DIM, 3 * CH], F32, isOutput=False)
    w2 = nc.declare_dram_parameter("w2", [CH, DIM], F32, isOutput=False)
    bias = nc.declare_dram_parameter("bias", [1, DIM], F32, isOutput=False)
    ec = nc.declare_dram_parameter("ec", [8, CH], F32, isOutput=False)
    y = nc.declare_dram_parameter("y", [T, DIM], F32, isOutput=True)

    with tile.TileContext(nc) as tc:
        with tc.tile_pool(name="persist", bufs=1) as pp:
            # ---- constants / persistent tiles ----
            ones_col = pp.tile([P, 1], F32, name="ones_col", tag="ones_col")
            nc.vector.memset(ones_col[:, :], 1.0)

            w1t = []
            for ct in range(8):
                t_ = pp.tile([P, 3 * CH], F32, name=f"w1_{ct}", tag=f"w1_{ct}")
                nc.sync.dma_start(out=t_[:, :], in_=w1[ct * P:(ct + 1) * P, :])
                w1t.append(t_)

            qt = [
                pp.tile([P, T], F32, name=f"qt_{j}", tag=f"qt_{j}")
                for j in range(4)
            ]

            kvt = [
                pp.tile([P, P], F32, name=f"kvt_{j}", tag=f"kvt_{j}")
                for j in range(4)
            ]
            zt = pp.tile([1, CH], F32, name="zt", tag="zt")

            phase_a(nc, tc, pp, T, NTB, xT, w1t, qt, kvt, zt, ones_col)
            phase_b(nc, tc, pp, T, NTB, w2, bias, ec, y, qt, kvt, zt)

    nc.compile()
    return nc


def phase_a(nc, tc, pp, T, NTB, xT, w1t, qt, kvt, zt, ones_col):
    with (
        tc.tile_pool(name="phA_sb", bufs=2) as pa,
        tc.tile_pool(name="xload", bufs=12) as xp,
        tc.tile_pool(name="proj_ps", bufs=3, space="PSUM") as proj_ps,
        tc.tile_pool(name="hold_ps", bufs=1, space="PSUM") as hold_ps,
    ):
            # PSUM accumulators held across all of phase A
            kvps = [
                hold_ps.tile([P, P], F32, name=f"kvps_{j}", tag=f"kvps_{j}")
                for j in range(4)
            ]
            zps = hold_ps.tile([1, CH], F32, name="zps", tag="zps")

            # ---- phase A: projections + kv/z accumulation ----
            for ib in range(NTB):
                tsl = slice(ib * 512, (ib + 1) * 512)
                xt = []
                for ct in range(8):
                    t_ = xp.tile([P, 512], F32, name=f"xt_{ib}_{ct}", tag="xt")
                    nc.sync.dma_start(out=t_[:, :], in_=xT[ct * P:(ct + 1) * P, tsl])
                    xt.append(t_)

                # q projection (c-major) with elu+1, into persistent qt
                for j in range(4):
                    qps = proj_ps.tile([P, 512], F32, name=f"qps_{ib}_{j}", tag="proj")
                    for ct in range(8):
                        nc.tensor.matmul(
                            qps[:, :],
                            w1t[ct][:, j * P:(j + 1) * P],
                            xt[ct][:, :],
                            start=(ct == 0),
                            stop=(ct == 7),
                        )
                    m_ = pa.tile([P, 512], F32, name=f"qm_{ib}_{j}", tag="elu_m")
                    e_ = pa.tile([P, 512], F32, name=f"qe_{ib}_{j}", tag="elu_e")
                    r_ = pa.tile([P, 512], F32, name=f"qr_{ib}_{j}", tag="elu_r")
                    nc.vector.tensor_scalar_min(m_[:, :], qps[:, :], 0.0)
                    nc.scalar.activation(e_[:, :], m_[:, :], AF.Exp)
                    nc.scalar.activation(r_[:, :], qps[:, :], AF.Relu)
                    nc.vector.tensor_add(qt[j][:, tsl], e_[:, :], r_[:, :])

                # k, v projections (token-major) per 128-token block
                for t in range(4):
                    tok = slice(t * P, (t + 1) * P)
                    kps = proj_ps.tile([P, 512], F32, name=f"kps_{ib}_{t}", tag="proj")
                    for ct in range(8):
                        nc.tensor.matmul(
                            kps[:, :],
                            xt[ct][:, tok],
                            w1t[ct][:, CH:2 * CH],
                            start=(ct == 0),
                            stop=(ct == 7),
                        )
                    km = pa.tile([P, 512], F32, name=f"km_{ib}_{t}", tag="elu_m")
                    ke = pa.tile([P, 512], F32, name=f"ke_{ib}_{t}", tag="elu_e")
                    kr = pa.tile([P, 512], F32, name=f"kr_{ib}_{t}", tag="elu_r")
                    k_sb = pa.tile([P, 512], F32, name=f"k_{ib}_{t}", tag="k_sb")
                    nc.vector.tensor_scalar_min(km[:, :], kps[:, :], 0.0)
                    nc.scalar.activation(ke[:, :], km[:, :], AF.Exp)
                    nc.scalar.activation(kr[:, :], kps[:, :], AF.Relu)
                    nc.vector.tensor_add(k_sb[:, :], ke[:, :], kr[:, :])

                    vps = proj_ps.tile([P, 512], F32, name=f"vps_{ib}_{t}", tag="proj")
                    for ct in range(8):
                        nc.tensor.matmul(
                            vps[:, :],
                            xt[ct][:, tok],
                            w1t[ct][:, 2 * CH:3 * CH],
                            start=(ct == 0),
                            stop=(ct == 7),
                        )
                    v_sb = pa.tile([P, 512], F32, name=f"v_{ib}_{t}", tag="v_sb")
                    nc.vector.tensor_copy(v_sb[:, :], vps[:, :])

                    first = (ib == 0 and t == 0)
                    last = (ib == NTB - 1 and t == 3)
                    # z += ones^T k   [1, 512]
                    nc.tensor.matmul(
                        zps[0:1, :], ones_col[:, :], k_sb[:, :],
                        start=first, stop=last, skip_group_check=True,
                    )
                    # kvT[j] += v_pair^T k_pair   [128, 128] per head-pair
                    for j in range(4):
                        csl = slice(j * P, (j + 1) * P)
                        nc.tensor.matmul(
                            kvps[j][:, :], v_sb[:, csl], k_sb[:, csl],
                            start=first, stop=last, skip_group_check=True,
                        )

            # ---- evict PSUM accumulators before releasing phase-A pools ----
            for j in range(4):
                nc.vector.memset(kvt[j][:, :], 0.0)
                nc.vector.tensor_copy(kvt[j][0:64, 0:64], kvps[j][0:64, 0:64])
                nc.vector.tensor_copy(
                    kvt[j][64:128, 64:128], kvps[j][64:128, 64:128]
                )
            nc.vector.tensor_copy(zt[0:1, :], zps[0:1, :])


def phase_b(nc, tc, pp, T, NTB, w2, bias, ec, y, qt, kvt, zt):
            # ---- phase B setup: Zblock, E, W2, bias, Mstack ----
            Zb, Es = [], []
            for j in range(4):
                zb = pp.tile([P, 8], F32, name=f"Zb_{j}", tag=f"Zb_{j}")
                nc.vector.memset(zb[:, :], 0.0)
                nc.sync.dma_start(
                    out=zb[0:64, 2 * j:2 * j + 1],
                    in_=zt[0:1, j * P:j * P + 64],
                )
                nc.sync.dma_start(
                    out=zb[64:128, 2 * j + 1:2 * j + 2],
                    in_=zt[0:1, j * P + 64:(j + 1) * P],
                )
                Zb.append(zb)
            ec_sb = pp.tile([8, CH], F32, name="ec_sb", tag="ec_sb")
            nc.sync.dma_start(out=ec_sb[:, :], in_=ec[:, :])
            for j in range(4):
                Es.append(ec_sb[:, j * P:(j + 1) * P])

            w2t = []
            for j in range(4):
                t_ = pp.tile([P, DIM], F32, name=f"w2_{j}", tag=f"w2_{j}")
                nc.sync.dma_start(out=t_[:, :], in_=w2[j * P:(j + 1) * P, :])
                w2t.append(t_)

            b_sb = pp.tile([1, DIM], F32, name="b_sb", tag="b_sb")
            nc.sync.dma_start(out=b_sb[0:1, :], in_=bias[0:1, :])
            ones_row = pp.tile([1, P], F32, name="ones_row", tag="ones_row")
            nc.vector.memset(ones_row[:, :], 1.0)
            b_bc = pp.tile([P, DIM], F32, name="b_bc", tag="b_bc")

            with (
                tc.tile_pool(name="phB_sb", bufs=2) as pb,
                tc.tile_pool(name="qsc_pool", bufs=8) as qp,
                tc.tile_pool(name="phB_ps", bufs=2, space="PSUM") as bps,
                tc.tile_pool(name="y_ps", bufs=2, space="PSUM") as yps_pool,
            ):
                for h in range(2):
                    hsl = slice(h * 512, (h + 1) * 512)
                    bb_ps = bps.tile([P, 512], F32, name=f"bbps_{h}", tag="m")
                    nc.tensor.matmul(
                        bb_ps[:, :], ones_row[0:1, :], b_sb[0:1, hsl],
                        start=True, stop=True,
                    )
                    nc.vector.tensor_copy(b_bc[:, hsl], bb_ps[:, :])

                Ms = []
                for j in range(4):
                    ms = pp.tile([P, DIM], F32, name=f"Ms_{j}", tag=f"Ms_{j}")
                    for h in range(2):
                        hsl = slice(h * 512, (h + 1) * 512)
                        mps = bps.tile([P, 512], F32, name=f"mps_{j}_{h}", tag="m")
                        nc.tensor.matmul(
                            mps[:, :], kvt[j][:, :], w2t[j][:, hsl],
                            start=True, stop=True,
                        )
                        nc.vector.tensor_copy(ms[:, hsl], mps[:, :])
                    Ms.append(ms)

                # ---- phase B main: den -> r -> broadcast -> qsc -> y ----
                for ib in range(NTB):
                    tsl = slice(ib * 512, (ib + 1) * 512)
                    dps = bps.tile([8, 512], F32, name=f"dps_{ib}", tag="d")
                    for j in range(4):
                        nc.tensor.matmul(
                            dps[:, :], Zb[j][:, :], qt[j][:, tsl],
                            start=(j == 0), stop=(j == 3),
                        )
                    rT = pb.tile([8, 512], F32, name=f"rT_{ib}", tag="rT")
                    nc.vector.tensor_scalar_add(rT[:, :], dps[:, :], 1e-6)
                    nc.vector.reciprocal(rT[:, :], rT[:, :])

                    qsc = []
                    for j in range(4):
                        bcp = bps.tile([P, 512], F32, name=f"bcp_{ib}_{j}", tag="bc")
                        nc.tensor.matmul(
                            bcp[:, :], Es[j][:, :], rT[:, :],
                            start=True, stop=True,
                        )
                        qs = qp.tile([P, 512], F32, name=f"qsc_{ib}_{j}", tag="qsc")
                        nc.vector.tensor_mul(qs[:, :], qt[j][:, tsl], bcp[:, :])
                        qsc.append(qs)

                    for t in range(4):
                        tok = slice(t * P, (t + 1) * P)
                        y_sb = pb.tile([P, DIM], F32, name=f"y_{ib}_{t}", tag="y_sb", bufs=3)
                        for h in range(2):
                            hsl = slice(h * 512, (h + 1) * 512)
                            yp = yps_pool.tile(
                                [P, 512], F32, name=f"yps_{ib}_{t}_{h}", tag="y"
                            )
                            for j in range(4):
                                nc.tensor.matmul(
                                    yp[:, :], qsc[j][:, tok], Ms[j][:, hsl],
                                    start=(j == 0), stop=(j == 3),
                                )
                            nc.vector.tensor_add(y_sb[:, hsl], yp[:, :], b_bc[:, hsl])
                        row = (ib * 4 + t) * P
                        nc.sync.dma_start(out=y[row:row + P, :], in_=y_sb[:, :])


_NC_CACHE = {}


def _get_nc(T=T_FULL):
    if T not in _NC_CACHE:
        _NC_CACHE[T] = build_nc(T)
    return _NC_CACHE[T]


def make_in_maps(x, W_qkv, W_out, b_out):
    x = np.asarray(x, dtype=np.float32)
    W_qkv = np.asarray(W_qkv, dtype=np.float32)
    W_out = np.asarray(W_out, dtype=np.float32)
    b_out = np.asarray(b_out, dtype=np.float32)

    xTs = [np.ascontiguousarray(x[b].T) for b in range(B)]
    w1s, w2s = [], []
    for hg in range(2):
        cs = slice(hg * CH, (hg + 1) * CH)
        w1s.append(
            np.ascontiguousarray(
                np.concatenate(
                    [W_qkv[:, cs],
                     W_qkv[:, DIM + hg * CH:DIM + (hg + 1) * CH],
                     W_qkv[:, 2 * DIM + hg * CH:2 * DIM + (hg + 1) * CH]],
                    axis=1,
                )
            )
        )
        w2s.append(np.ascontiguousarray(W_out[cs, :]))
    bh = np.ascontiguousarray((b_out * 0.5).reshape(1, DIM))
    ecm = make_ec()

    in_maps = []
    for core in range(N_CORES):
        b, hg = core // 2, core % 2
        in_maps.append(
            {"xT": xTs[b], "w1": w1s[hg], "w2": w2s[hg], "bias": bh, "ec": ecm}
        )
    return in_maps


def make_ec():
    """E selector: ec[h, j*128+p] = 1 iff head-of-partition-p-in-tile-j == h."""
    ecm = np.zeros((8, CH), dtype=np.float32)
    for j in range(4):
        ecm[2 * j, j * P:j * P + 64] = 1.0
        ecm[2 * j + 1, j * P + 64:(j + 1) * P] = 1.0
    return ecm


def kernel(x, W_qkv, W_out, b_out):
    from concourse.bass_utils import run_bass_kernel_spmd

    nc = _get_nc(T_FULL)
    in_maps = make_in_maps(x, W_qkv, W_out, b_out)
    res = run_bass_kernel_spmd(nc, in_maps, core_ids=list(range(N_CORES))).results
    out = np.empty((B, T_FULL, DIM), dtype=np.float32)
    for b in range(B):
        out[b] = res[2 * b]["y"] + res[2 * b + 1]["y"]
    return out


# revision 10
# speedup vs baseline: 3.1747x; 3.1747x over previous
"""Linear attention kernel for 8 Trainium2 NeuronCores.

Sharding: core = 2*b + hg  (b in 0..3 batches, hg in 0..1 head-groups of 8 heads).
Fully data-parallel — no collectives; host sums the two head-group partials per
batch. Each core adds bias/2 so the pair-sum carries the full bias.

Per-core math (T=4096 tokens, CH=512 = 8 heads x 64, DIM=1024):
  qT   = (x @ Wq)^T          c-major [CH, T], elu+1
  k,v  = x @ Wk, x @ Wv      token-major [T, CH], elu+1 on k
  kvT  = v^T k  (per head-pair, diagonal 64-blocks valid), accumulated in PSUM
  z    = ones^T k            [1, CH], accumulated in PSUM
  den  = Zblock^T qT         [8, T]   (Zblock = block-diag expansion of z)
  r    = 1/(den + 1e-6); rbc = E^T r  (broadcast r over each head's 64 rows)
  qsc  = qT * rbc
  M    = kvT^T @ W2  (per 128-row ch-tile; off-diag blocks of kvT zeroed)
  y    = qsc^T @ M + bias/2  token-major [T, DIM]
"""

import sys

sys.path.insert(0, "/opt/trn_rl_repo")

import numpy as np

import concourse.bass as bass
import concourse.mybir as mybir
import concourse.tile as tile
from concourse import bacc

F32 = mybir.dt.float32
BF16 = mybir.dt.bfloat16
AF = mybir.ActivationFunctionType

DIM = 1024      # model dim (contraction for projections)
CH = 512        # per-core channels (8 heads x 64)
P = 128

N_CORES = 8
B, T_FULL = 4, 4096


def build_nc(T=T_FULL):
    NTB = T // 512          # 512-token blocks
    nc = bacc.Bacc(None, target_bir_lowering=False, debug=False)

    xT = nc.declare_dram_parameter("xT", [DIM, T], BF16, isOutput=False)
    w1 = nc.declare_dram_parameter("w1", [DIM, 3 * CH], BF16, isOutput=False)
    w2 = nc.declare_dram_parameter("w2", [CH, DIM], BF16, isOutput=False)
    bias = nc.declare_dram_parameter("bias", [1, DIM], BF16, isOutput=False)
    ec = nc.declare_dram_parameter("ec", [8, CH], BF16, isOutput=False)
    y = nc.declare_dram_parameter("y", [T, DIM], F32, isOutput=True)

    with tile.TileContext(nc) as tc:
        with tc.tile_pool(name="persist", bufs=1) as pp:
            # ---- constants / persistent tiles ----
            ones_col = pp.tile([P, 1], BF16, name="ones_col", tag="ones_col")
            nc.vector.memset(ones_col[:, :], 1.0)

            w1t = []
            for ct in range(8):
                t_ = pp.tile([P, 3 * CH], BF16, name=f"w1_{ct}", tag=f"w1_{ct}")
                nc.sync.dma_start(out=t_[:, :], in_=w1[ct * P:(ct + 1) * P, :])
                w1t.append(t_)

            qt = [
                pp.tile([P, T], BF16, name=f"qt_{j}", tag=f"qt_{j}")
                for j in range(4)
            ]

            kvt = [
                pp.tile([P, P], BF16, name=f"kvt_{j}", tag=f"kvt_{j}")
                for j in range(4)
            ]
            zt = pp.tile([1, CH], BF16, name="zt", tag="zt")

            phase_a(nc, tc, pp, T, NTB, xT, w1t, qt, kvt, zt, ones_col)
            phase_b(nc, tc, pp, T, NTB, w2, bias, ec, y, qt, kvt, zt)

    nc.compile()
    return nc


def phase_a(nc, tc, pp, T, NTB, xT, w1t, qt, kvt, zt, ones_col):
    with (
        tc.tile_pool(name="phA_sb", bufs=2) as pa,
        tc.tile_pool(name="xload", bufs=12) as xp,
        tc.tile_pool(name="proj_ps", bufs=3, space="PSUM") as proj_ps,
        tc.tile_pool(name="hold_ps", bufs=1, space="PSUM") as hold_ps,
    ):
            # PSUM accumulators held across all of phase A
            kvps = [
                hold_ps.tile([P, P], F32, name=f"kvps_{j}", tag=f"kvps_{j}")
                for j in range(4)
            ]
            zps = hold_ps.tile([1, CH], F32, name="zps", tag="zps")

            # ---- phase A: projections + kv/z accumulation ----
            for ib in range(NTB):
                tsl = slice(ib * 512, (ib + 1) * 512)
                xt = []
                for ct in range(8):
                    t_ = xp.tile([P, 512], BF16, name=f"xt_{ib}_{ct}", tag="xt")
                    nc.sync.dma_start(out=t_[:, :], in_=xT[ct * P:(ct + 1) * P, tsl])
                    xt.append(t_)

                # q projection (c-major) with elu+1, into persistent qt
                for j in range(4):
                    qps = proj_ps.tile([P, 512], F32, name=f"qps_{ib}_{j}", tag="proj")
                    for ct in range(8):
                        nc.tensor.matmul(
                            qps[:, :],
                            w1t[ct][:, j * P:(j + 1) * P],
                            xt[ct][:, :],
                            start=(ct == 0),
                            stop=(ct == 7),
                        )
                    m_ = pa.tile([P, 512], F32, name=f"qm_{ib}_{j}", tag="elu_m")
                    e_ = pa.tile([P, 512], F32, name=f"qe_{ib}_{j}", tag="elu_e")
                    r_ = pa.tile([P, 512], F32, name=f"qr_{ib}_{j}", tag="elu_r")
                    nc.vector.tensor_scalar_min(m_[:, :], qps[:, :], 0.0)
                    nc.scalar.activation(e_[:, :], m_[:, :], AF.Exp)
                    nc.scalar.activation(r_[:, :], qps[:, :], AF.Relu)
                    nc.vector.tensor_add(qt[j][:, tsl], e_[:, :], r_[:, :])

                # k, v projections (token-major) per 128-token block
                for t in range(4):
                    tok = slice(t * P, (t + 1) * P)
                    kps = proj_ps.tile([P, 512], F32, name=f"kps_{ib}_{t}", tag="proj")
                    for ct in range(8):
                        nc.tensor.matmul(
                            kps[:, :],
                            xt[ct][:, tok],
                            w1t[ct][:, CH:2 * CH],
                            start=(ct == 0),
                            stop=(ct == 7),
                        )
                    km = pa.tile([P, 512], F32, name=f"km_{ib}_{t}", tag="elu_m")
                    ke = pa.tile([P, 512], F32, name=f"ke_{ib}_{t}", tag="elu_e")
                    kr = pa.tile([P, 512], F32, name=f"kr_{ib}_{t}", tag="elu_r")
                    k_sb = pa.tile([P, 512], BF16, name=f"k_{ib}_{t}", tag="k_sb")
                    nc.vector.tensor_scalar_min(km[:, :], kps[:, :], 0.0)
                    nc.scalar.activation(ke[:, :], km[:, :], AF.Exp)
                    nc.scalar.activation(kr[:, :], kps[:, :], AF.Relu)
                    nc.vector.tensor_add(k_sb[:, :], ke[:, :], kr[:, :])

                    vps = proj_ps.tile([P, 512], F32, name=f"vps_{ib}_{t}", tag="proj")
                    for ct in range(8):
                        nc.tensor.matmul(
                            vps[:, :],
                            xt[ct][:, tok],
                            w1t[ct][:, 2 * CH:3 * CH],
                            start=(ct == 0),
                            stop=(ct == 7),
                        )
                    v_sb = pa.tile([P, 512], BF16, name=f"v_{ib}_{t}", tag="v_sb")
                    nc.vector.tensor_copy(v_sb[:, :], vps[:, :])

                    first = (ib == 0 and t == 0)
                    last = (ib == NTB - 1 and t == 3)
                    # z += ones^T k   [1, 512]
                    nc.tensor.matmul(
                        zps[0:1, :], ones_col[:, :], k_sb[:, :],
                        start=first, stop=last, skip_group_check=True,
                    )
                    # kvT[j] += v_pair^T k_pair   [128, 128] per head-pair
                    for j in range(4):
                        csl = slice(j * P, (j + 1) * P)
                        nc.tensor.matmul(
                            kvps[j][:, :], v_sb[:, csl], k_sb[:, csl],
                            start=first, stop=last, skip_group_check=True,
                        )

            # ---- evict PSUM accumulators before releasing phase-A pools ----
            for j in range(4):
                nc.vector.memset(kvt[j][:, :], 0.0)
                nc.vector.tensor_copy(kvt[j][0:64, 0:64], kvps[j][0:64, 0:64])
                nc.vector.tensor_copy(
                    kvt[j][64:128, 64:128], kvps[j][64:128, 64:128]
                )
            nc.vector.tensor_copy(zt[0:1, :], zps[0:1, :])


def phase_b(nc, tc, pp, T, NTB, w2, bias, ec, y, qt, kvt, zt):
            # ---- phase B setup: Zblock, E, W2, bias, Mstack ----
            Zb, Es = [], []
            for j in range(4):
                zb = pp.tile([P, 8], BF16, name=f"Zb_{j}", tag=f"Zb_{j}")
                nc.vector.memset(zb[:, :], 0.0)
                nc.sync.dma_start(
                    out=zb[0:64, 2 * j:2 * j + 1],
                    in_=zt[0:1, j * P:j * P + 64],
                )
                nc.sync.dma_start(
                    out=zb[64:128, 2 * j + 1:2 * j + 2],
                    in_=zt[0:1, j * P + 64:(j + 1) * P],
                )
                Zb.append(zb)
            ec_sb = pp.tile([8, CH], BF16, name="ec_sb", tag="ec_sb")
            nc.sync.dma_start(out=ec_sb[:, :], in_=ec[:, :])
            for j in range(4):
                Es.append(ec_sb[:, j * P:(j + 1) * P])

            w2t = []
            for j in range(4):
                t_ = pp.tile([P, DIM], BF16, name=f"w2_{j}", tag=f"w2_{j}")
                nc.sync.dma_start(out=t_[:, :], in_=w2[j * P:(j + 1) * P, :])
                w2t.append(t_)

            b_sb = pp.tile([1, DIM], BF16, name="b_sb", tag="b_sb")
            nc.sync.dma_start(out=b_sb[0:1, :], in_=bias[0:1, :])
            ones_row = pp.tile([1, P], BF16, name="ones_row", tag="ones_row")
            nc.vector.memset(ones_row[:, :], 1.0)
            b_bc = pp.tile([P, DIM], F32, name="b_bc", tag="b_bc")

            with (
                tc.tile_pool(name="phB_sb", bufs=2) as pb,
                tc.tile_pool(name="qsc_pool", bufs=8) as qp,
                tc.tile_pool(name="phB_ps", bufs=2, space="PSUM") as bps,
                tc.tile_pool(name="y_ps", bufs=2, space="PSUM") as yps_pool,
            ):
                for h in range(2):
                    hsl = slice(h * 512, (h + 1) * 512)
                    bb_ps = bps.tile([P, 512], F32, name=f"bbps_{h}", tag="m")
                    nc.tensor.matmul(
                        bb_ps[:, :], ones_row[0:1, :], b_sb[0:1, hsl],
                        start=True, stop=True,
                    )
                    nc.vector.tensor_copy(b_bc[:, hsl], bb_ps[:, :])

                Ms = []
                for j in range(4):
                    ms = pp.tile([P, DIM], BF16, name=f"Ms_{j}", tag=f"Ms_{j}")
                    for h in range(2):
                        hsl = slice(h * 512, (h + 1) * 512)
                        mps = bps.tile([P, 512], F32, name=f"mps_{j}_{h}", tag="m")
                        nc.tensor.matmul(
                            mps[:, :], kvt[j][:, :], w2t[j][:, hsl],
                            start=True, stop=True,
                        )
                        nc.vector.tensor_copy(ms[:, hsl], mps[:, :])
                    Ms.append(ms)

                # ---- phase B main: den -> r -> broadcast -> qsc -> y ----
                for ib in range(NTB):
                    tsl = slice(ib * 512, (ib + 1) * 512)
                    dps = bps.tile([8, 512], F32, name=f"dps_{ib}", tag="d")
                    for j in range(4):
                        nc.tensor.matmul(
                            dps[:, :], Zb[j][:, :], qt[j][:, tsl],
                            start=(j == 0), stop=(j == 3),
                        )
                    rf = pb.tile([8, 512], F32, name=f"rf_{ib}", tag="rf")
                    nc.vector.tensor_scalar_add(rf[:, :], dps[:, :], 1e-6)
                    nc.vector.reciprocal(rf[:, :], rf[:, :])
                    rT = pb.tile([8, 512], BF16, name=f"rT_{ib}", tag="rT")
                    nc.vector.tensor_copy(rT[:, :], rf[:, :])

                    qsc = []
                    for j in range(4):
                        bcp = bps.tile([P, 512], F32, name=f"bcp_{ib}_{j}", tag="bc")
                        nc.tensor.matmul(
                            bcp[:, :], Es[j][:, :], rT[:, :],
                            start=True, stop=True,
                        )
                        qs = qp.tile([P, 512], BF16, name=f"qsc_{ib}_{j}", tag="qsc")
                        nc.vector.tensor_mul(qs[:, :], qt[j][:, tsl], bcp[:, :])
                        qsc.append(qs)

                    for t in range(4):
                        tok = slice(t * P, (t + 1) * P)
                        y_sb = pb.tile([P, DIM], F32, name=f"y_{ib}_{t}", tag="y_sb", bufs=3)
                        for h in range(2):
                            hsl = slice(h * 512, (h + 1) * 512)
                            yp = yps_pool.tile(
                                [P, 512], F32, name=f"yps_{ib}_{t}_{h}", tag="y"
                            )
                            for j in range(4):
                                nc.tensor.matmul(
                                    yp[:, :], qsc[j][:, tok], Ms[j][:, hsl],
                                    start=(j == 0), stop=(j == 3),
                                )
                            nc.vector.tensor_add(y_sb[:, hsl], yp[:, :], b_bc[:, hsl])
                        row = (ib * 4 + t) * P
                        nc.sync.dma_start(out=y[row:row + P, :], in_=y_sb[:, :])


_NC_CACHE = {}


def _get_nc(T=T_FULL):
    if T not in _NC_CACHE:
        _NC_CACHE[T] = build_nc(T)
    return _NC_CACHE[T]


def make_in_maps(x, W_qkv, W_out, b_out):
    import ml_dtypes

    bf16 = ml_dtypes.bfloat16
    x = np.asarray(x, dtype=np.float32)
    W_qkv = np.asarray(W_qkv, dtype=np.float32).astype(bf16)
    W_out = np.asarray(W_out, dtype=np.float32).astype(bf16)
    b_out = np.asarray(b_out, dtype=np.float32)

    xTs = [np.ascontiguousarray(x[b].T.astype(bf16)) for b in range(B)]
    w1s, w2s = [], []
    for hg in range(2):
        cs = slice(hg * CH, (hg + 1) * CH)
        w1s.append(
            np.ascontiguousarray(
                np.concatenate(
                    [W_qkv[:, cs],
                     W_qkv[:, DIM + hg * CH:DIM + (hg + 1) * CH],
                     W_qkv[:, 2 * DIM + hg * CH:2 * DIM + (hg + 1) * CH]],
                    axis=1,
                )
            )
        )
        w2s.append(np.ascontiguousarray(W_out[cs, :]))
    bh = np.ascontiguousarray((b_out * 0.5).reshape(1, DIM).astype(bf16))
    ecm = make_ec().astype(bf16)

    in_maps = []
    for core in range(N_CORES):
        b, hg = core // 2, core % 2
        in_maps.append(
            {"xT": xTs[b], "w1": w1s[hg], "w2": w2s[hg], "bias": bh, "ec": ecm}
        )
    return in_maps


def make_ec():
    """E selector: ec[h, j*128+p] = 1 iff head-of-partition-p-in-tile-j == h."""
    ecm = np.zeros((8, CH), dtype=np.float32)
    for j in range(4):
        ecm[2 * j, j * P:j * P + 64] = 1.0
        ecm[2 * j + 1, j * P + 64:(j + 1) * P] = 1.0
    return ecm


def kernel(x, W_qkv, W_out, b_out):
    from concourse.bass_utils import run_bass_kernel_spmd

    nc = _get_nc(T_FULL)
    in_maps = make_in_maps(x, W_qkv, W_out, b_out)
    res = run_bass_kernel_spmd(nc, in_maps, core_ids=list(range(N_CORES))).results
    out = np.empty((B, T_FULL, DIM), dtype=np.float32)
    for b in range(B):
        out[b] = res[2 * b]["y"] + res[2 * b + 1]["y"]
    return out
